# revision 17
# baseline (speedup 1.0000x reference)
"""GAU (Gated Attention Unit) Trainium2 Bass kernel, 8-core sequence-parallel.

Reference computation (all fp32):
    hid  = silu(x @ W_hidden + b_hidden);  v, gate = split(hid, 2)
    qk   = silu(x @ W_qk + b_qk)
    q    = qk * gamma[0] + beta[0];  k = qk * gamma[1] + beta[1]
    attn = relu((q @ k.T) / sqrt(dim))^2
    out  = ((attn @ v) * gate) @ W_out + b_out
    return out * x

Sharding (v6): rows (N=8192) split across 8 cores, 1024 rows each. Each core
computes k / v / q / gate for its OWN rows only, then AllGathers k and v in
ONE collective (bf16 k + fp8 v packed into a single byte buffer) — one
skew/floor payment per rep, and regular DMA stalls against an in-flight AG
are paid once. Phase-1 runs qk first (it needs no DMA beyond the persistent
x tiles, so it fills the tail of the previous rep), then v, then the AG
trigger, then the gate, which is produced and consumed entirely in SBUF so
it drains without DMA while the AG is in flight. gpsimd carries only the
collective trigger.

Phase 2 computes attn for BOTH 512-row i-chunks (k tiles read once), then
one fp8-DoubleRow attn@v pass (256-deep contraction, 2x PE rate) reading
each v tile once, then the W_out contraction. All PSUM cycles through one
"acc" tag of 4-bank slots, double-buffered.

Precision: phase-1 matmuls bf16; k/q bf16; v and attn fp8e4 (E4M3), attn
scaled by 4 (q by 2) with W_out/4 compensation. Measured end-to-end max rel
err 6.2e-3 against the fp32 reference (gate: 2e-2). fp8 anywhere else
(phase-1 x/W, sim q/k, gated) was numerically checked and exceeds the gate.

reps are pipelined (no global barrier); x tiles / W_qk load once up front;
small stage tiles live in persistent pools so successive reps overlap.
"""

import numpy as np

import concourse.bass as bass
import concourse.mybir as mybir
import concourse.tile as tile
from concourse import bacc

N = 8192          # total rows
D = 1024          # model dim
QK = 200          # qk dim
H = 2048          # hidden (v/gate) dim
NC = 8            # cores
R = N // NC       # rows per core
DT = D // 128     # d-tiles
HT = H // 128     # h-tiles
IC = R // 512     # i-chunks per core (own j-groups of 512)
JT = N // 128     # total j-tiles

f32 = mybir.dt.float32
f32r = mybir.dt.float32r
bf16 = mybir.dt.bfloat16
fp8 = mybir.dt.float8e4
u8 = mybir.dt.uint8
ACT = mybir.ActivationFunctionType
ALU = mybir.AluOpType
DR = mybir.MatmulPerfMode.DoubleRow
RG = [list(range(NC))]

KB_K = 2 * 128 * R * 2          # kT (bf16) bytes in the combined AG buffer
KB_V = R * H                    # v (fp8) bytes
KB_ALL = KB_K + KB_V


def _build_nc(reps=1, vbias=False, obias=False,
              do_p1=True, do_pA=True, do_pB=True, do_pC=True):
    nc = bacc.Bacc("TRN2", target_bir_lowering=False, debug=False,
                   num_devices=NC)

    xT_own = nc.dram_tensor("xT_own", [D, R], bf16, kind="ExternalInput").ap()
    x_own = nc.dram_tensor("x_own", [R, D], f32, kind="ExternalInput").ap()
    w_h = nc.dram_tensor("w_h", [D, 2 * H], bf16, kind="ExternalInput").ap()
    w_qk = nc.dram_tensor("w_qk", [D, QK], bf16, kind="ExternalInput").ap()
    w_out = nc.dram_tensor("w_out", [H, D], bf16, kind="ExternalInput").ap()
    # per-c scalars, padded 200 -> [2, 128]
    gq = nc.dram_tensor("gq", [2, 128], f32, kind="ExternalInput").ap()
    bq = nc.dram_tensor("bq", [2, 128], f32, kind="ExternalInput").ap()
    gk = nc.dram_tensor("gk", [2, 128], f32, kind="ExternalInput").ap()
    bk = nc.dram_tensor("bk", [2, 128], f32, kind="ExternalInput").ap()
    bqk = nc.dram_tensor("bqk", [2, 128], f32, kind="ExternalInput").ap()
    bg = nc.dram_tensor("bg", [HT, 128], f32, kind="ExternalInput").ap()
    if vbias:
        bv = nc.dram_tensor("bv", [H], f32, kind="ExternalInput").ap()
    if obias:
        bo = nc.dram_tensor("bo", [D], f32, kind="ExternalInput").ap()
    out = nc.dram_tensor("out", [R, D], f32, kind="ExternalOutput").ap()

    with tile.TileContext(nc) as tc:
        with (
            tc.tile_pool(name="pers", bufs=1) as pers,
            tc.tile_pool(name="stp", bufs=3) as stp,
            tc.tile_pool(name="dram", bufs=1, space="DRAM") as dpool,
        ):
            # persistent small tiles
            gq_t = pers.tile([128, 2], f32)
            bq_t = pers.tile([128, 2], f32)
            gk_t = pers.tile([128, 2], f32)
            bk_t = pers.tile([128, 2], f32)
            bqk_t = pers.tile([128, 2], f32)
            bg_t = pers.tile([128, HT], f32)
            nc.sync.dma_start(out=gq_t, in_=gq.rearrange("ct c -> c ct"))
            nc.sync.dma_start(out=bq_t, in_=bq.rearrange("ct c -> c ct"))
            nc.sync.dma_start(out=gk_t, in_=gk.rearrange("ct c -> c ct"))
            nc.sync.dma_start(out=bk_t, in_=bk.rearrange("ct c -> c ct"))
            nc.sync.dma_start(out=bqk_t, in_=bqk.rearrange("ct c -> c ct"))
            nc.sync.dma_start(out=bg_t, in_=bg.rearrange("ht c -> c ht"))
            if vbias:
                bv_t = pers.tile([128, H], f32)
                nc.sync.dma_start(
                    out=bv_t,
                    in_=bass.AP(tensor=bv.tensor, offset=bv.offset,
                                ap=[[0, 128]] + list(bv.ap)),
                )
            if obias:
                bo_t = pers.tile([128, D], f32)
                nc.sync.dma_start(
                    out=bo_t,
                    in_=bass.AP(tensor=bo.tensor, offset=bo.offset,
                                ap=[[0, 128]] + list(bo.ap)),
                )

            # DRAM scratch: one combined (k + v) AG buffer per direction.
            # Shared AG outputs allow a single writer each -> one per rep.
            comb_own = dpool.tile([KB_ALL], u8, tag="comb_own")
            comb_ags = [dpool.tile([NC, KB_ALL], u8, tag=f"comb_ag{r}",
                                   name=f"comb_ag{r}", addr_space="Shared")
                        for r in range(reps)]
            kT_own = comb_own[0:KB_K].bitcast(bf16).rearrange(
                "(ct c j) -> ct c j", ct=2, c=128)
            v_own = comb_own[KB_K:KB_ALL].bitcast(fp8).rearrange(
                "(r h) -> r h", h=H)

            def k_ag(ag, rank):          # [2, 128, R] bf16 view of rank's k
                return ag[rank, 0:KB_K].bitcast(bf16).rearrange(
                    "(ct c j) -> ct c j", ct=2, c=128)

            def v_ag(ag, rank):          # [R, H] fp8 view of rank's v
                return ag[rank, KB_K:KB_ALL].bitcast(fp8).rearrange(
                    "(r h) -> r h", h=H)

            # qT lives in SBUF for the whole kernel (1 MB)
            qT_s = pers.tile([128, 2, R], bf16, tag="qT_s", name="qT_s")

            xT_r = xT_own.rearrange("(dt p) (jg j) -> p dt jg j", p=128, j=512)
            wh_r = w_h.rearrange("(dt p) h -> p dt h", p=128)
            wqk_r = w_qk.rearrange("(dt p) c -> p dt c", p=128)
            wo_r = w_out.rearrange("(ht p) m -> p ht m", p=128)
            xo_r = x_own.rearrange("(it p) m -> p it m", p=128)

            # rep-invariant SBUF: x tiles + W_qk, loaded once
            wqk_t = pers.tile([128, DT, QK], bf16, tag="wqk", name="wqk_t")
            nc.sync.dma_start(out=wqk_t, in_=wqk_r)
            xg = pers.tile([128, DT, IC, 512], bf16, tag="xg", name="xg")
            for jg in range(IC):
                for dh in range(2):
                    eng = nc.sync if (jg + dh) % 2 else nc.scalar
                    eng.dma_start(
                        out=xg[:, dh * 4:(dh + 1) * 4, jg, :],
                        in_=xT_r[:, dh * 4:(dh + 1) * 4, jg, :])

            for rep in range(reps):
                comb_ag = comb_ags[rep]
                # gate lives in SBUF for the whole rep (written by phase 1,
                # read by phase 2's B drain) - no DMA on the gate path
                with tc.tile_pool(name="grp", bufs=1) as grp:
                    gT_s = grp.tile([128, HT, R], bf16, tag="gT_s",
                                    name="gT_s")
                    # ============ phase 1: own-row v, k/q, gate + AG ============
                    if do_p1:
                     with (
                        tc.tile_pool(name="whp", bufs=1) as whp,
                        tc.tile_pool(name="ps_qk", bufs=2, space="PSUM") as ps_qk,
                        tc.tile_pool(name="ps_v", bufs=2, space="PSUM") as ps_v,
                        tc.tile_pool(name="ps_g", bufs=2, space="PSUM") as ps_g,
                    ):
                        wh_t = whp.tile([128, DT, 2 * H], bf16, tag="wh")
                        for dt in range(DT):
                            eng = nc.sync if dt % 2 else nc.scalar
                            eng.dma_start(out=wh_t[:, dt, :], in_=wh_r[:, dt, :])

                        # ---- qk -> k + q (own rows) ----
                        for jg in range(IC):
                            for ct in range(2):
                                cw = 128 if ct == 0 else QK - 128
                                pq = ps_qk.tile([128, 512], f32)
                                for dt in range(DT):
                                    nc.tensor.matmul(
                                        pq[:cw],
                                        wqk_t[:, dt, ct * 128:ct * 128 + cw],
                                        xg[:, dt, jg, :],
                                        start=(dt == 0),
                                        stop=(dt == DT - 1),
                                    )
                                sil = stp.tile([128, 512], f32, tag="sil",
                                               name="sil")
                                nc.scalar.activation(
                                    sil[:cw], pq[:cw], ACT.Silu,
                                    bias=bqk_t[:cw, ct:ct + 1],
                                )
                                kt = stp.tile([128, 512], bf16, tag="kt",
                                              name="kt")
                                nc.vector.tensor_scalar(
                                    out=kt[:cw], in0=sil[:cw],
                                    scalar1=gk_t[:cw, ct:ct + 1],
                                    scalar2=bk_t[:cw, ct:ct + 1],
                                    op0=ALU.mult, op1=ALU.add,
                                )
                                nc.sync.dma_start(
                                    out=kT_own[ct, 0:cw,
                                               jg * 512:(jg + 1) * 512],
                                    in_=kt[:cw],
                                )
                                nc.vector.tensor_scalar(
                                    out=qT_s[:cw, ct, jg * 512:(jg + 1) * 512],
                                    in0=sil[:cw],
                                    scalar1=gq_t[:cw, ct:ct + 1],
                                    scalar2=bq_t[:cw, ct:ct + 1],
                                    op0=ALU.mult, op1=ALU.add,
                                )
                        # ---- v (own rows, row-major fp8) ----
                        for jg in range(IC):
                            for jt in range(4):
                                for hc in range(4):
                                    pv = ps_v.tile([128, 512], f32)
                                    for dt in range(DT):
                                        nc.tensor.matmul(
                                            pv,
                                            xg[:, dt, jg, jt * 128:(jt + 1) * 128],
                                            wh_t[:, dt, hc * 512:(hc + 1) * 512],
                                            start=(dt == 0),
                                            stop=(dt == DT - 1),
                                        )
                                    vt = stp.tile([128, 512], fp8, tag="vt",
                                                  name="vt")
                                    if vbias:
                                        tmp = stp.tile([128, 512], f32,
                                                       tag="vtmp", name="vtmp")
                                        nc.vector.tensor_add(
                                            tmp, pv,
                                            bv_t[:, hc * 512:(hc + 1) * 512])
                                        nc.scalar.activation(vt, tmp, ACT.Silu)
                                    else:
                                        nc.scalar.activation(vt, pv, ACT.Silu)
                                    veng = nc.sync if (jt + hc) % 2 else nc.scalar
                                    veng.dma_start(
                                        out=v_own[(jg * 4 + jt) * 128:
                                                  (jg * 4 + jt + 1) * 128,
                                                  hc * 512:(hc + 1) * 512],
                                        in_=vt,
                                    )

                        # k + v slices written -> ONE AllGather for both
                        nc.gpsimd.collective_compute(
                            "AllGather", ALU.bypass, replica_groups=RG,
                            ins=[comb_own.opt()], outs=[comb_ag.opt()],
                        )

                        # ---- gateT (own rows) -> SBUF only ----
                        for jg in range(IC):
                            for ht in range(HT):
                                pg = ps_g.tile([128, 512], f32)
                                for dt in range(DT):
                                    nc.tensor.matmul(
                                        pg,
                                        wh_t[:, dt,
                                             H + ht * 128:H + (ht + 1) * 128],
                                        xg[:, dt, jg, :],
                                        start=(dt == 0),
                                        stop=(dt == DT - 1),
                                    )
                                nc.scalar.activation(
                                    gT_s[:, ht, jg * 512:(jg + 1) * 512],
                                    pg, ACT.Silu, bias=bg_t[:, ht:ht + 1])

                    # ========= phase 2: attention, all PSUM via one tag =========
                    # "acc" slots are [128, 2048] fp32 = 4 banks, bufs=2.
                    with (
                        tc.tile_pool(name="p2sb", bufs=1) as p2sb,
                        tc.tile_pool(name="kqp", bufs=2) as kqp,
                        tc.tile_pool(name="vst", bufs=4) as vst,
                        tc.tile_pool(name="wop", bufs=2) as wop,
                        tc.tile_pool(name="xop", bufs=4) as xop,
                        tc.tile_pool(name="ost", bufs=2) as osp,
                        tc.tile_pool(name="p2ps", bufs=2, space="PSUM") as p2ps,
                    ):
                        # attn (x4-scaled, fp8) for BOTH i-chunks
                        attn = p2sb.tile([128, JT, R], fp8, tag="attn",
                                         name="attn")
                        gated = p2sb.tile([128, HT, R], bf16, tag="gated",
                                          name="gated")

                        # ---- A: attn = relu(2 k.T q)^2; k tiles read once ----
                        if do_pA:
                            for jg in range(JT // 4):
                                kt_sb = kqp.tile([128, 2, 512], bf16,
                                                 tag="kt_sb", name="kt_sb")
                                keng = nc.sync if jg % 2 else nc.scalar
                                keng.dma_start(
                                    out=kt_sb,
                                    in_=k_ag(comb_ag, jg // 2)
                                    [:, :, (jg % 2) * 512:(jg % 2 + 1) * 512]
                                    .rearrange("ct c j -> c ct j"),
                                )
                                for ic in range(IC):
                                    q_sb = qT_s[:, :, ic * 512:(ic + 1) * 512]
                                    pss = p2ps.tile([128, 4, 512], f32,
                                                    tag="acc", name="pss")
                                    for j4 in range(4):
                                        nc.tensor.matmul(
                                            pss[:, j4, :],
                                            kt_sb[:, 0,
                                                  j4 * 128:(j4 + 1) * 128],
                                            q_sb[:, 0, :],
                                            start=True, stop=False)
                                        nc.tensor.matmul(
                                            pss[:, j4, :],
                                            kt_sb[0:QK - 128, 1,
                                                  j4 * 128:(j4 + 1) * 128],
                                            q_sb[0:QK - 128, 1, :],
                                            start=False, stop=True)
                                    rel = kqp.tile([128, 4, 512], bf16,
                                                   tag="rel", name="rel")
                                    nc.scalar.activation(rel, pss, ACT.Relu)
                                    nc.vector.tensor_mul(
                                        attn[:, 4 * jg:4 * jg + 4,
                                             ic * 512:(ic + 1) * 512],
                                        rel, rel)

                            if not (do_pB and do_pC):
                                pa = kqp.tile([128, 512], f32, tag="pa",
                                              bufs=1, name="pa")
                                nc.vector.tensor_copy(pa, attn[:, 0, 0:512])
                                nc.sync.dma_start(
                                    out=out.rearrange("(a p) m -> p a m",
                                                      p=128)
                                    [:, 1, 0:512], in_=pa)

                        # ---- B: out1T = v-lhsT @ attn (fp8 DoubleRow), *gate ----
                        if do_pB:
                            for hg in range(HT // 2):
                                po = p2ps.tile([128, 2, IC, 512], f32,
                                               tag="acc", name="po")
                                for jt2 in range(JT // 2):
                                    vt = vst.tile([128, 2, 256], fp8,
                                                  tag="vt", name="vt")
                                    veng = nc.sync if jt2 % 2 else nc.scalar
                                    veng.dma_start(
                                        out=vt,
                                        in_=v_ag(comb_ag, jt2 // 4)
                                        [(jt2 % 4) * 256:(jt2 % 4 + 1) * 256,
                                         hg * 256:(hg + 1) * 256]
                                        .rearrange("(ko k) h -> k ko h",
                                                   k=128),
                                    )
                                    for hh in range(2):
                                        for ic2 in range(IC):
                                            nc.tensor.matmul(
                                                po[:, hh, ic2, :],
                                                vt[:, :,
                                                   hh * 128:(hh + 1) * 128],
                                                attn[:, 2 * jt2:2 * jt2 + 2,
                                                     ic2 * 512:(ic2 + 1) * 512],
                                                start=(jt2 == 0),
                                                stop=(jt2 == JT // 2 - 1),
                                                perf_mode=DR,
                                            )
                                for hh in range(2):
                                    ht = hg * 2 + hh
                                    nc.vector.tensor_mul(
                                        gated[:, ht, :],
                                        po[:, hh, :, :].rearrange(
                                            "p a b -> p (a b)"),
                                        gT_s[:, ht, :])

                            if not do_pC:
                                pb = kqp.tile([128, 512], f32, tag="pb",
                                              bufs=1, name="pb")
                                nc.vector.tensor_copy(pb, gated[:, 0, 0:512])
                                nc.sync.dma_start(
                                    out=out.rearrange("(a p) m -> p a m",
                                                      p=128)
                                    [:, 4, 0:512], in_=pb)

                        # ---- C: out2 = gatedT.T @ (W_out/4); out = out2 * x ----
                        if do_pC:
                            for mc in range(2):
                                pos0 = p2ps.tile([128, 4, 512], f32,
                                                 tag="acc", name="pos0")
                                pos1 = p2ps.tile([128, 4, 512], f32,
                                                 tag="acc", name="pos1")
                                poss = (pos0, pos1)
                                # prefetch the x halves for this mc
                                xos = []
                                for it in range(8):
                                    xo = xop.tile([128, 512], f32, tag="xo",
                                                  name="xo")
                                    xeng = nc.sync if it % 2 else nc.scalar
                                    xeng.dma_start(
                                        out=xo,
                                        in_=xo_r[:, it,
                                                 mc * 512:(mc + 1) * 512])
                                    xos.append(xo)
                                for hq in range(4):
                                    wo = wop.tile([128, 4, 512], bf16,
                                                  tag="wo", name="wo")
                                    for dh in range(2):
                                        eng = (nc.sync if (hq + dh) % 2
                                               else nc.scalar)
                                        eng.dma_start(
                                            out=wo[:, dh * 2:(dh + 1) * 2, :],
                                            in_=wo_r[:, hq * 4 + dh * 2:
                                                     hq * 4 + (dh + 1) * 2,
                                                     mc * 512:(mc + 1) * 512],
                                        )
                                    for h4 in range(4):
                                        for it in range(8):
                                            nc.tensor.matmul(
                                                poss[it // 4][:, it % 4, :],
                                                gated[:, hq * 4 + h4,
                                                      it * 128:(it + 1) * 128],
                                                wo[:, h4, :],
                                                start=(hq == 0 and h4 == 0),
                                                stop=(hq == 3 and h4 == 3),
                                            )
                                for it in range(8):
                                    ot = osp.tile([128, 512], f32, tag="ot",
                                                  name="ot")
                                    if obias:
                                        nc.vector.tensor_add(
                                            ot, poss[it // 4][:, it % 4, :],
                                            bo_t[:, mc * 512:(mc + 1) * 512])
                                        nc.vector.tensor_mul(ot, ot, xos[it])
                                    else:
                                        nc.vector.tensor_mul(
                                            ot, poss[it // 4][:, it % 4, :],
                                            xos[it])
                                    oeng = nc.sync if it % 2 else nc.scalar
                                    oeng.dma_start(
                                        out=out.rearrange(
                                            "(it p) m -> p it m", p=128)
                                        [:, it, mc * 512:(mc + 1) * 512],
                                        in_=ot,
                                    )

            # anchor outputs for phase-subset timing builds (prevents DCE)
            if not (do_pA and do_pB and do_pC):
                tc.strict_bb_all_engine_barrier()
                with tc.tile_pool(name="probe", bufs=1) as prp:
                    if do_p1:
                        pt = prp.tile([128, 512], f32)
                        nc.sync.dma_start(
                            out=pt,
                            in_=comb_ags[-1][0, 0:256 * 1024]
                            .bitcast(f32).rearrange("(p a) -> p a", p=128)
                            [:, 0:512])
                        nc.sync.dma_start(
                            out=out.rearrange("(a p) m -> p a m", p=128)
                            [:, 0, 0:512], in_=pt)

    nc.compile()
    return nc


# ---------------------------------------------------------------- runner ----

import time as _time

import jax
import jax.numpy as jnp
from jax.sharding import Mesh, NamedSharding, PartitionSpec
from jax.experimental.shard_map import shard_map

from concourse.bass2jax import _bass_exec_p, install_neuronx_cc_hook, partition_id_tensor


class SpmdRunner:
    def __init__(self, nc, n_cores=8):
        install_neuronx_cc_hook()
        self.nc = nc
        self.n_cores = n_cores
        partition_name = nc.partition_id_tensor.name if nc.partition_id_tensor else None
        in_names, out_names, out_avals, zero_outs = [], [], [], []
        for alloc in nc.m.functions[0].allocations:
            if not isinstance(alloc, mybir.MemoryLocationSet):
                continue
            name = alloc.memorylocations[0].name
            if alloc.kind == "ExternalInput":
                if name != partition_name:
                    in_names.append(name)
            elif alloc.kind == "ExternalOutput":
                shape = tuple(alloc.tensor_shape)
                dtype = mybir.dt.np(alloc.dtype)
                out_names.append(name)
                out_avals.append(jax.core.ShapedArray(shape, dtype))
                zero_outs.append(np.zeros(shape, dtype))
        self.in_names, self.out_names = in_names, out_names
        self.out_avals, self.zero_outs = out_avals, zero_outs
        n_params = len(in_names)
        all_names = in_names + out_names
        if partition_name is not None:
            all_names = all_names + [partition_name]

        def _body(*args):
            operands = list(args)
            if partition_name is not None:
                operands.append(partition_id_tensor())
            outs = _bass_exec_p.bind(
                *operands,
                out_avals=tuple(out_avals),
                in_names=tuple(all_names),
                out_names=tuple(out_names),
                lowering_input_output_aliases=(),
                sim_require_finite=True,
                sim_require_nnan=True,
                nc=nc,
            )
            return tuple(outs)

        devices = jax.devices()[:n_cores]
        self.mesh = Mesh(np.asarray(devices), ("core",))
        in_specs = (PartitionSpec("core"),) * (n_params + len(out_names))
        out_specs = (PartitionSpec("core"),) * len(out_names)
        self.sharded = jax.jit(
            shard_map(_body, mesh=self.mesh, in_specs=in_specs,
                      out_specs=out_specs, check_rep=False),
            keep_unused=True,
        )

    def stage_inputs(self, in_maps):
        n = self.n_cores
        concat = [
            np.concatenate([np.asarray(in_maps[c][name]) for c in range(n)], axis=0)
            for name in self.in_names
        ]
        concat += [np.zeros((n * z.shape[0], *z.shape[1:]), z.dtype)
                   for z in self.zero_outs]
        sharding = NamedSharding(self.mesh, PartitionSpec("core"))
        return [jax.device_put(a, sharding) for a in concat]

    def run(self, staged):
        outs = self.sharded(*staged)
        jax.block_until_ready(outs)
        return outs

    def run_numpy(self, staged):
        outs = self.run(staged)
        n = self.n_cores
        return [
            {name: np.asarray(outs[i]).reshape(n, *self.out_avals[i].shape)[c]
             for i, name in enumerate(self.out_names)}
            for c in range(n)
        ]


# ------------------------------------------------------------- host side ----

_CACHE = {}


def _get_runner(reps, vbias, obias):
    key = (reps, vbias, obias)
    if key not in _CACHE:
        nc = _build_nc(reps=reps, vbias=vbias, obias=obias)
        _CACHE[key] = SpmdRunner(nc, NC)
    return _CACHE[key]


def _pad2(v):
    o = np.zeros((2, 128), np.float32)
    o[0] = v[:128]
    o[1, :QK - 128] = v[128:QK]
    return o


# attn is computed as relu(2*sim)^2 = 4*relu(sim)^2 to center E4M3;
# compensated by staging W_out/4.
ATTN_SCALE = 2.0


def make_in_maps(x, W_hidden, b_hidden, W_qk, b_qk, gamma, beta, W_out, b_out):
    bf16_np = mybir.dt.np(bf16)
    x = np.ascontiguousarray(np.asarray(x, np.float32))
    scale = ATTN_SCALE / np.sqrt(np.float32(D))
    gq = _pad2(np.asarray(gamma[0], np.float32) * scale)
    bq = _pad2(np.asarray(beta[0], np.float32) * ATTN_SCALE)
    gk = _pad2(np.asarray(gamma[1], np.float32))
    bk = _pad2(np.asarray(beta[1], np.float32))
    bqk = _pad2(np.asarray(b_qk, np.float32))
    bg = np.ascontiguousarray(
        np.asarray(b_hidden[H:], np.float32).reshape(HT, 128))
    W_hidden = np.ascontiguousarray(
        np.asarray(W_hidden, np.float32).astype(bf16_np))
    W_qk = np.ascontiguousarray(
        np.asarray(W_qk, np.float32).astype(bf16_np))
    W_out = np.ascontiguousarray(
        (np.asarray(W_out, np.float32) / (ATTN_SCALE * ATTN_SCALE))
        .astype(bf16_np))
    bv = np.asarray(b_hidden[:H], np.float32)
    bo = np.asarray(b_out, np.float32)
    vbias = bool(np.any(bv))
    obias = bool(np.any(bo))

    xT = np.ascontiguousarray(x.T)
    in_maps = []
    for c in range(NC):
        m = {
            "xT_own": np.ascontiguousarray(
                xT[:, c * R:(c + 1) * R]).astype(bf16_np),
            "x_own": x[c * R:(c + 1) * R],
            "w_h": W_hidden,
            "w_qk": W_qk,
            "w_out": W_out,
            "gq": gq, "bq": bq, "gk": gk, "bk": bk, "bqk": bqk, "bg": bg,
        }
        if vbias:
            m["bv"] = bv
        if obias:
            m["bo"] = bo
        in_maps.append(m)
    return in_maps, vbias, obias


def kernel(x, W_hidden, b_hidden, W_qk, b_qk, gamma, beta, W_out, b_out):
    in_maps, vbias, obias = make_in_maps(
        x, W_hidden, b_hidden, W_qk, b_qk, gamma, beta, W_out, b_out)
    runner = _get_runner(1, vbias, obias)
    staged = runner.stage_inputs(in_maps)
    results = runner.run_numpy(staged)
    return np.concatenate([results[c]["out"] for c in range(NC)], axis=0)
